# revision 1
# baseline (speedup 1.0000x reference)
"""Continuous-time RNN kernel for Trainium2 (8 NeuronCores, Bass/Tile).

Math (per reference):
    ih    = x @ W_ih.T + b_ih                     # time-invariant drive
    decay = exp(-dt / tau),  dt = 0.1
    10x:  h = decay * h + (1 - decay) * tanh(ih + h @ W_hh.T + b_hh)

Strategy:
  - Data-parallel over batch: 4096 rows -> 8 cores x 512.
  - State kept transposed on-chip: hT [H=2048 partdim-chunks, B=512 free].
    Matmuls use the weight chunk as the stationary operand and hT as the
    moving operand, so no transposes are needed inside the recurrence.
  - Matmul operands in bf16 (4x PE throughput vs fp32), accumulation and
    the decay blend in fp32.  Measured end-to-end rel err ~3e-3 absmax.
  - PSUM: one bank per output chunk j ([128,512] fp32), accumulate over
    16 k-chunks, evacuate via DVE add of the precomputed ih+biases term,
    tanh on ScalarE, blend on DVE/ScalarE.
"""

import numpy as np
import ml_dtypes

H = 2048
I = 1024
B_TOTAL = 4096
N_CORES = 8
B = B_TOTAL // N_CORES  # 512 per-core batch shard
KJ = H // 128  # 16 output/contraction chunks of the hidden dim
KI = I // 128  # 8 contraction chunks of the input dim
NUM_STEPS = 10
DT = 0.1

_NC_CACHE = {}


def _build_nc():
    import concourse.mybir as mybir
    import concourse.tile as tile
    from concourse import bacc

    f32 = mybir.dt.float32
    bf16 = mybir.dt.bfloat16
    Tanh = mybir.ActivationFunctionType.Tanh

    nc = bacc.Bacc(None, target_bir_lowering=False, debug=False)

    x_t = nc.declare_dram_parameter("x_t", [I, B], bf16, isOutput=False)
    h0f = nc.declare_dram_parameter("h0f", [H, B], f32, isOutput=False)
    wih = nc.declare_dram_parameter("wih", [I, H], bf16, isOutput=False)
    whh = nc.declare_dram_parameter("whh", [H, H], bf16, isOutput=False)
    # packed per-partition vectors: [decay | 1-decay | b_ih+b_hh], each [128, KJ]
    vecs = nc.declare_dram_parameter("vecs", [128, 3 * KJ], f32, isOutput=False)
    hout = nc.declare_dram_parameter("hout", [H, B], f32, isOutput=True)

    # whh is host-permuted to per-j column slabs: row j*128+p, col k*128+q
    # holds W_hh[j*128+q, k*128+p]; slab j is one contiguous [128, H] DMA.
    whh_r = whh[:].rearrange("(j p) f -> j p f", p=128)
    wih_r = wih[:].rearrange("(k p) j -> k p j", p=128)
    xt_r = x_t[:].rearrange("(i p) b -> p i b", p=128)  # [128, KI, B]
    h0f_r = h0f[:].rearrange("(k p) b -> k p b", p=128)
    ho_r = hout[:].rearrange("(k p) b -> k p b", p=128)

    with tile.TileContext(nc) as tc:
        with (
            tc.tile_pool(name="whhp", bufs=1) as whhp,
            tc.tile_pool(name="xp", bufs=1) as xp,
            tc.tile_pool(name="hfp", bufs=1) as hfp,
            tc.tile_pool(name="hbp", bufs=1) as hbp,
            tc.tile_pool(name="ihbp", bufs=1) as ihbp,
            tc.tile_pool(name="vecp", bufs=1) as vecp,
            tc.tile_pool(name="ps", bufs=8, space="PSUM") as ps,
        ):
            vec_t = vecp.tile([128, 3 * KJ], f32, name="vec_t")

            # NOTE: do NOT add PE "warmup" matmuls to pre-trip the HAM clock
            # gate — measured three ways, any early PE activity collapses the
            # SWDGE queue carrying the first weight chunk to ~60GB/s and
            # delays the real start by far more than the cold-clock penalty.

            def dec(j):
                return vec_t[:, j : j + 1]

            def omd(j):
                return vec_t[:, KJ + j : KJ + j + 1]

            def bsm(j):
                return vec_t[:, 2 * KJ + j : 2 * KJ + j + 1]

            # Emission order sets DMA priority: phase-0's operands (x, W_ih)
            # first so PE starts ~immediately; then slab 0 of W_hh, h0, and
            # the remaining W_hh slabs, which stream in behind phase 0 and
            # are consumed per-j as the recurrence's first step progresses.
            Xt = xp.tile([128, KI, B], bf16, name="x_all")
            IHB = [ihbp.tile([128, B], f32, name=f"ihb_{k}") for k in range(KJ)]

            with tc.tile_pool(name="wihp", bufs=1) as wihp:
                WI = [
                    wihp.tile([128, H], bf16, name=f"wih_{i}") for i in range(KI)
                ]
                # first matmul is gated on wih chunk 0 + x chunks 0-3 only;
                # wih0 goes via gpsimd (SWDGE) in parallel with sync's x DMA
                nc.gpsimd.dma_start(out=WI[0][:], in_=wih_r[0, :, :])
                nc.sync.dma_start(out=Xt[:, 0 : KI // 2, :], in_=xt_r[:, 0 : KI // 2, :])
                nc.sync.dma_start(out=Xt[:, KI // 2 :, :], in_=xt_r[:, KI // 2 :, :])
                nc.gpsimd.dma_start(out=vec_t[:], in_=vecs[:])
                for i in range(1, KI):
                    nc.sync.dma_start(out=WI[i][:], in_=wih_r[i, :, :])

                # W_hh slab 0 + h0 + remaining slabs (stream during phase 0)
                W = []
                for j in range(KJ):
                    w = whhp.tile([128, H], bf16, name=f"whh_{j}")
                    W.append(w)
                nc.sync.dma_start(out=W[0][:], in_=whh_r[0, :, :])
                HF, HB0, HB1 = [], [], []
                for k in range(KJ):
                    hf = hfp.tile([128, B], f32, name=f"hf_{k}")
                    nc.sync.dma_start(out=hf[:], in_=h0f_r[k, :, :])
                    HF.append(hf)
                    hb = hbp.tile([128, B], bf16, name=f"hb0_{k}")
                    nc.vector.tensor_copy(out=hb[:], in_=hf[:])  # fp32 -> bf16
                    HB0.append(hb)
                    HB1.append(hbp.tile([128, B], bf16, name=f"hb1_{k}"))
                for j in range(1, KJ):
                    nc.sync.dma_start(out=W[j][:], in_=whh_r[j, :, :])

                # ---- phase 0: ihb = x @ W_ih.T + (b_ih + b_hh), transposed.
                # Two halves of 8 PSUM banks; interleaved accumulation groups
                # across banks are bank-independent.
                for jh in range(2):
                    psums = []
                    for jj in range(8):
                        p0 = ps.tile([128, B], f32, name=f"p0_{jh}_{jj}", tag="bank")
                        psums.append(p0)
                    for i in range(KI):
                        for jj in range(8):
                            j = jh * 8 + jj
                            nc.tensor.matmul(
                                psums[jj][:],
                                WI[i][:, j * 128 : (j + 1) * 128],
                                Xt[:, i, :],
                                start=(i == 0),
                                stop=(i == KI - 1),
                            )
                    for jj in range(8):
                        j = jh * 8 + jj
                        nc.vector.tensor_scalar_add(
                            out=IHB[j][:], in0=psums[jj][:], scalar1=bsm(j)
                        )

            # ---- recurrence: 10 steps
            with tc.tile_pool(name="scr", bufs=2) as scr:
                cur, nxt = HB0, HB1
                for t in range(NUM_STEPS):
                    for j in range(KJ):
                        pp = ps.tile([128, B], f32, name=f"pp_{t}_{j}", tag="bank")
                        for k in range(KJ):
                            nc.tensor.matmul(
                                pp[:],
                                W[j][:, k * 128 : (k + 1) * 128],
                                cur[k][:],
                                start=(k == 0),
                                stop=(k == KJ - 1),
                            )
                        # last step: evacuate in B/2 halves to shorten the
                        # post-last-matmul serial chain (and store per half)
                        halves = (
                            [(0, B)]
                            if t < NUM_STEPS - 1
                            else [(0, B // 2), (B // 2, B)]
                        )
                        m1 = scr.tile([128, B], f32, name=f"m1_{t}_{j}", tag="m1")
                        nc.scalar.mul(out=m1[:], in_=HF[j][:], mul=dec(j))
                        for h0_, h1_ in halves:
                            hs = slice(h0_, h1_)
                            pre = scr.tile(
                                [128, B], f32, name=f"pre_{t}_{j}", tag="pre"
                            )
                            nc.vector.tensor_add(
                                out=pre[:, hs], in0=pp[:, hs], in1=IHB[j][:, hs]
                            )
                            tgt = scr.tile(
                                [128, B], f32, name=f"tgt_{t}_{j}", tag="tgt"
                            )
                            nc.scalar.activation(
                                out=tgt[:, hs], in_=pre[:, hs], func=Tanh
                            )
                            m2 = scr.tile([128, B], f32, name=f"m2_{t}_{j}", tag="m2")
                            nc.vector.tensor_scalar_mul(
                                out=m2[:, hs], in0=tgt[:, hs], scalar1=omd(j)
                            )
                            nc.vector.tensor_add(
                                out=HF[j][:, hs], in0=m1[:, hs], in1=m2[:, hs]
                            )
                            if t < NUM_STEPS - 1:
                                nc.vector.tensor_copy(out=nxt[j][:], in_=HF[j][:])
                            else:
                                nc.sync.dma_start(
                                    out=ho_r[j, :, hs], in_=HF[j][:, hs]
                                )
                    cur, nxt = nxt, cur

    nc.compile()
    return nc


def _get_nc():
    if "nc" not in _NC_CACHE:
        _NC_CACHE["nc"] = _build_nc()
    return _NC_CACHE["nc"]


def _host_prep(x, h0, W_ih, b_ih, W_hh, b_hh, tau):
    bf = ml_dtypes.bfloat16
    f32 = np.float32

    decay = np.exp(f32(-DT) / np.asarray(tau, f32)).astype(f32)
    omd = (f32(1.0) - decay).astype(f32)
    bsum = (np.asarray(b_ih, f32) + np.asarray(b_hh, f32)).astype(f32)

    vecs = np.zeros((128, 3 * KJ), f32)
    vecs[:, 0:KJ] = decay.reshape(KJ, 128).T
    vecs[:, KJ : 2 * KJ] = omd.reshape(KJ, 128).T
    vecs[:, 2 * KJ : 3 * KJ] = bsum.reshape(KJ, 128).T

    wih_b = np.ascontiguousarray(np.asarray(W_ih, f32).T).astype(bf)  # [I, H]
    # per-j column slabs: row j*128+p, col k*128+q = W_hh[j*128+q, k*128+p]
    whh_b = np.ascontiguousarray(
        np.asarray(W_hh, f32)
        .reshape(KJ, 128, KJ, 128)
        .transpose(0, 3, 2, 1)
        .reshape(H, H)
    ).astype(bf)

    in_maps = []
    for c in range(N_CORES):
        xs = np.asarray(x[c * B : (c + 1) * B], f32)
        hs = np.asarray(h0[c * B : (c + 1) * B], f32)
        xT = np.ascontiguousarray(xs.T).astype(bf)  # [I, B]
        hT = np.ascontiguousarray(hs.T)  # [H, B] fp32
        in_maps.append(
            {"x_t": xT, "h0f": hT, "wih": wih_b, "whh": whh_b, "vecs": vecs}
        )
    return in_maps


def kernel(x, h0, W_ih, b_ih, W_hh, b_hh, tau):
    from concourse.bass_utils import run_bass_kernel_spmd

    x, h0, W_ih, b_ih, W_hh, b_hh, tau = (
        np.asarray(a) for a in (x, h0, W_ih, b_ih, W_hh, b_hh, tau)
    )
    assert x.shape == (B_TOTAL, I) and h0.shape == (B_TOTAL, H)
    nc = _get_nc()
    in_maps = _host_prep(x, h0, W_ih, b_ih, W_hh, b_hh, tau)
    res = run_bass_kernel_spmd(nc, in_maps, list(range(N_CORES)))
    out = np.empty((B_TOTAL, H), np.float32)
    for c in range(N_CORES):
        out[c * B : (c + 1) * B] = np.asarray(res.results[c]["hout"], np.float32).T
    return out

